# revision 14
# baseline (speedup 1.0000x reference)
"""ConvLSTM cell (complex-valued gates) on 8 TRN2 NeuronCores.

Strategy
--------
Data-parallel over batch: 16 images -> 2 per core. Per core, each gate's
complex 3x3 conv uses Gauss's 3-multiplication trick instead of the
naive 4:

    k1 = (Wr+Wi) (*) zr,  k2 = Wr (*) (zi-zr),  k3 = Wi (*) (zr+zi)
    yr = k1 - k3,         yi = k1 + k2

which cuts tensor-engine streaming time 25%. Per 3x3 tap each term is a
[128in -> out] matmul accumulated in PSUM over the 9 taps:

  - gates i+o pack their 64 output channels into M=128 matmuls
    (PSUM banks A=[k1_i|k1_o], B=[k3_i|k3_o], C=[k2_i|k2_o]); the
    combines yr_io=A-B / yi_io=A+C are same-base full-width vector ops.
  - gate c is M=64; row-blocks are processed in PAIRS (m0, m1) and the
    two macros' accumulators share banks D/E/F at partitions 0:64 /
    64:128, written by column-tiled matmuls that run concurrently in
    the PE array (tile_position derives from the psum base partition).
    yrc=D-E / yic=D+F again combine both macros in one op each.

All matmul operands are fp16. ScalarE applies sigmoid/tanh with the
per-channel bias fused (reads may cross partition bases). VectorE does
the complex elementwise update in fp16. x (*) c_prev is precomputed on
the host and added on-chip. z streams (zr, zi-zr, zr+zi) are zero-padded
to 66x66 on the host so conv taps are plain shifted access patterns.
"""
import sys
import numpy as np

sys.path.insert(0, "/opt/trn_rl_repo")

P = 128          # partitions / channels (64 real + 64 imag)
HALF = 64
B = 16           # full batch
N_CORES = 8
B_CORE = B // N_CORES   # batch per core
H = W = 64
HP = WP = 66     # padded spatial
RB = 8           # rows per macro tile
COLS = RB * W    # 512 columns per macro tile / PSUM bank
NSLOT = 27       # 9 taps x 3 gauss terms per weight pack

_CACHE = {}


def _apply_drain_patch(tile_mod):
    """The kernel-tail drain aggregates one wait per live proc-semaphore, but
    walrus rejects instructions with more than a few sync waits. Split the
    tail waits across a chain of single-wait drains."""
    if getattr(tile_mod.TileContext, "_drain_patched", False):
        return

    def _patched(self, tick_clock, wait_clock):
        ScopedClock = tile_mod.ScopedClock
        nc = self.nc
        drain_inst = nc.sync.drain()
        wait_clock.add_sem_waits(
            drain_inst.ins, ScopedClock({None: tick_clock.global_clock})
        )
        NW = 3
        si = drain_inst.ins.sync_info
        if si is not None and si.on_wait and len(si.on_wait) > NW:
            conds = list(si.on_wait)
            si.on_wait = conds[:NW]
            rest = conds[NW:]
            while rest:
                extra = nc.sync.drain()
                esi = extra.ins.sync_info
                if esi is None:
                    import bass_rust
                    extra.ins.sync_info = bass_rust.SyncInfo(
                        on_wait=rest[:NW], on_update=[])
                else:
                    esi.on_wait = rest[:NW]
                rest = rest[NW:]

        nc.all_engine_barrier()
        assert self.sems is not None
        popped = nc._tile_sem_poison_stack.pop()
        assert popped is self._sem_poison
        nc.clear_and_free_semaphores(list(self.sems.allocated().values()))
        nc.all_engine_barrier()

    tile_mod.TileContext._drain_and_barrier = _patched
    tile_mod.TileContext._drain_patched = True


def _split_excess_waits(nc, max_waits=1):
    """walrus's per-instruction sync-wait slots are tight (1 for some ISA
    structs). Hoist excess waits into same-engine no-ops inserted directly
    before the instruction — identical semantics, per-engine order kept."""
    import concourse.mybir as mybir
    n_new = 0
    for fn in nc.m.functions:
        for bb in fn.blocks:
            il = bb.instructions
            out = []
            for inst in il:
                si = inst.sync_info
                if si is not None and si.on_wait and len(si.on_wait) > max_waits:
                    conds = list(si.on_wait)
                    si.on_wait = conds[:max_waits]
                    rest = conds[max_waits:]
                    for j in range(0, len(rest), max_waits):
                        nop = mybir.InstNoOp(
                            name=f"{inst.name}_w{j}",
                            sync_info=mybir.SyncInfo(
                                on_wait=rest[j:j + max_waits], on_update=[]),
                            bass_nofuse=True,
                            engine=inst.engine,
                        )
                        out.append(nop)
                        n_new += 1
                out.append(inst)
            if n_new:
                il[:] = out
    return n_new


def _build_program():
    import concourse.bass as bass
    import concourse.tile as tile
    from concourse import mybir
    from contextlib import ExitStack

    _apply_drain_patch(tile)
    fp16 = mybir.dt.float16
    f32 = mybir.dt.float32
    Sigmoid = mybir.ActivationFunctionType.Sigmoid
    Tanh = mybir.ActivationFunctionType.Tanh
    Copy = mybir.ActivationFunctionType.Copy

    nc = bass.Bass("TRN2", target_bir_lowering=False, debug=False)
    # z component streams: r = zr, s = zr+zi, d = zi-zr
    z_d = {c: nc.dram_tensor("z" + c, [P, B_CORE, HP, WP], fp16,
                             kind="ExternalInput").ap()
           for c in "rsd"}
    wio_d = nc.dram_tensor("wio", [P, NSLOT, P], fp16, kind="ExternalInput").ap()
    wc_d = nc.dram_tensor("wc", [P, NSLOT, HALF], fp16, kind="ExternalInput").ap()
    b_d = nc.dram_tensor("bias", [P, 5], f32, kind="ExternalInput").ap()
    xc_d = nc.dram_tensor("xc", [P, B_CORE, H, W], fp16, kind="ExternalInput").ap()
    h_d = nc.dram_tensor("h_out", [P, B_CORE, H, W], fp16, kind="ExternalOutput").ap()
    c_d = nc.dram_tensor("c_out", [P, B_CORE, H, W], fp16, kind="ExternalOutput").ap()

    # gauss term -> weight slot base and rhs component
    TERM = {'k1': (0, 'r'), 'k3': (9, 's'), 'k2': (18, 'd')}

    # padded-row chunks (overlapping): A=[0:18) B=[16:34) C=[32:66)
    Z_CHUNKS = {'A': (0, 18), 'B': (16, 18), 'C': (32, 34)}

    with tile.TileContext(nc) as tc, ExitStack() as ctx:
        const = ctx.enter_context(tc.tile_pool(name="const", bufs=1))
        z_ch = {}

        def load_z(b, ch, comps="rsd", after=None):
            row0, nr = Z_CHUNKS[ch]
            for comp in comps:
                t = const.tile([P, nr, WP], fp16, name=f"z{comp}_{b}_{ch}")
                dm = nc.sync.dma_start(t[:], z_d[comp][:, b, row0:row0 + nr, :])
                if after is not None:
                    # hold the transfer back until the anchor matmul retires so
                    # it can't steal HBM bandwidth from earlier-needed loads
                    tile.add_dep_helper(dm.ins, after,
                                        reason="defer non-critical z load")
                z_ch[(comp, b, ch)] = t

        # loads in first-consumption order: the first conv group (gate-c k1)
        # only needs zr.A and the k1 third of wc, ~0.45MB of DMA
        load_z(0, 'A', comps="r")
        wc_s = const.tile([P, NSLOT, HALF], fp16, name="wc")
        nc.sync.dma_start(wc_s[:, 0:9, :], wc_d[:, 0:9, :])
        load_z(0, 'A', comps="s")
        nc.sync.dma_start(wc_s[:, 9:NSLOT, :], wc_d[:, 9:NSLOT, :])
        bias_s = const.tile([P, 5], f32)
        nc.sync.dma_start(bias_s[:], b_d[:])
        wio_s = const.tile([P, NSLOT, P], fp16, name="wio")
        nc.sync.dma_start(wio_s[:], wio_d[:])
        load_z(0, 'A', comps="d")

        ps_A = ctx.enter_context(tc.tile_pool(name="ps_A", bufs=1, space="PSUM"))
        ps_B = ctx.enter_context(tc.tile_pool(name="ps_B", bufs=1, space="PSUM"))
        ps_C = ctx.enter_context(tc.tile_pool(name="ps_C", bufs=1, space="PSUM"))
        ps_D = ctx.enter_context(tc.tile_pool(name="ps_D", bufs=2, space="PSUM"))
        ps_E = ctx.enter_context(tc.tile_pool(name="ps_E", bufs=1, space="PSUM"))
        ps_F = ctx.enter_context(tc.tile_pool(name="ps_F", bufs=2, space="PSUM"))
        work = ctx.enter_context(tc.tile_pool(name="work", bufs=2))
        pwork = ctx.enter_context(tc.tile_pool(name="pwork", bufs=2))

        # keep the PE busy on throwaway matmuls while the first real loads
        # are in flight: the HAM clock-gate needs ~3.4us of sustained PE
        # activity to unthrottle 1.2 -> 2.4 GHz, and the first conv would
        # otherwise run entirely in the cold window
        warm = const.tile([P, 128], fp16, name="warm")
        nc.vector.memzero(warm[:])
        warm_ps = ps_D.tile([P, COLS], f32, tag="pD")
        for _ in range(45):
            nc.tensor.matmul(warm_ps[:, 0:128], warm[:], warm[:],
                             start=True, stop=True)

        last_mm = [None]

        def rhs_ap(b, ch, r0, comp, kh, kw):
            roff = Z_CHUNKS[ch][0]
            zt = z_ch[(comp, b, ch)]
            r = r0 + kh - roff
            return zt[:, r:r + RB, kw:kw + W]

        def conv_io(pool, term, b, ch, r0, tag):
            """M=128 conv accumulation for the (i,o) gate pair."""
            base, comp = TERM[term]
            pt = pool.tile([P, COLS], f32, tag=tag)
            for t in range(9):
                kh, kw = t // 3, t % 3
                mm = nc.tensor.matmul(
                    pt[:], wio_s[:, base + t, :],
                    rhs_ap(b, ch, r0, comp, kh, kw),
                    start=(t == 0), stop=(t == 8),
                )
                last_mm[0] = mm.ins
            return pt

        def conv_c(pool, term, b, ch, r0a, r0b, tag):
            """M=64 conv for gate c: macro m0 -> psum[0:64] (PE cols 0:64),
            macro m1 -> psum[64:128] (PE cols 64:128); the two matmuls of a
            tap run concurrently in the array (column tiling)."""
            base, comp = TERM[term]
            pt = pool.tile([P, COLS], f32, tag=tag)
            for t in range(9):
                kh, kw = t // 3, t % 3
                nc.tensor.matmul(
                    pt[0:HALF, :], wc_s[:, base + t, :],
                    rhs_ap(b, ch, r0a, comp, kh, kw),
                    start=(t == 0), stop=(t == 8),
                )
                mm = nc.tensor.matmul(
                    pt[HALF:P, :], wc_s[:, base + t, :],
                    rhs_ap(b, ch, r0b, comp, kh, kw),
                    start=(t == 0), stop=(t == 8),
                )
                last_mm[0] = mm.ins
            return pt

        def epilogue_frees(pA, pB, pC):
            """PSUM-freeing combines, emitted immediately after the ABC
            groups so the banks recycle with no PE stall. PSUM has a single
            DVE read port, so the shared k1 accumulator is staged to SBUF by
            ScalarE (which sits closer to PSUM) and each vector op reads only
            one PSUM operand."""
            As = work.tile([P, COLS], f32, tag="As")
            nc.scalar.activation(As[:], pA[:], Copy)
            yrio = work.tile([P, COLS], fp16, tag="yrio")  # [yr_i | yr_o]
            nc.vector.tensor_sub(yrio[:], As[:], pB[:])
            yiio = work.tile([P, COLS], fp16, tag="yiio")  # [yi_i | yi_o]
            nc.vector.tensor_add(yiio[:], As[:], pC[:])
            return yrio, yiio

        def macro_epilogue(b, r0, beta, yrc, yic, yrio, yiio):
            """Per-macro elementwise chain. beta = 0 (m0) or 64 (m1): which
            half of the pair-level gate-c tiles this macro's channels use."""
            sl_c = slice(beta, beta + HALF)

            # gate activations; ScalarE reads cross partition bases freely.
            # The swapped/negated layout twins (CTs, O2) are VectorE 1-input
            # copies — the same-base rule only binds TensorTensor's two
            # inputs — keeping ScalarE (the busiest engine) lean.
            CT = work.tile([P, COLS], fp16, tag="CT")      # [ctr; cti]
            nc.scalar.activation(CT[0:HALF, :], yrc[sl_c, :], Tanh,
                                 bias=bias_s[sl_c, 2:3])
            nc.scalar.activation(CT[HALF:P, :], yic[sl_c, :], Tanh,
                                 bias=bias_s[sl_c, 3:4])
            CTs = work.tile([P, COLS], fp16, tag="CTs")    # [cti; -ctr]
            nc.vector.tensor_copy(CTs[0:HALF, :], CT[HALF:P, :])
            nc.vector.tensor_scalar_mul(CTs[HALF:P, :], CT[0:HALF, :], -1.0)

            I = work.tile([P, COLS], fp16, tag="I")        # [ir; ii]
            nc.scalar.activation(I[0:HALF, :], yrio[0:HALF, :], Sigmoid,
                                 bias=bias_s[0:HALF, 0:1])
            nc.scalar.activation(I[HALF:P, :], yiio[0:HALF, :], Sigmoid,
                                 bias=bias_s[0:HALF, 1:2])

            # i (*) ct (complex): product halves written to base-0/base-64 so
            # every TensorTensor keeps same-base inputs
            P1 = work.tile([P, COLS], fp16, tag="P1")      # [ir*ctr ; ir*cti]
            nc.vector.tensor_mul(P1[0:HALF, :], I[0:HALF, :], CT[0:HALF, :])
            nc.vector.tensor_mul(P1[HALF:P, :], I[0:HALF, :], CTs[0:HALF, :])
            P2 = work.tile([P, COLS], fp16, tag="P2")      # [ii*cti ; -ii*ctr]
            nc.vector.tensor_mul(P2[0:HALF, :], I[HALF:P, :], CT[HALF:P, :])
            nc.vector.tensor_mul(P2[HALF:P, :], I[HALF:P, :], CTs[HALF:P, :])
            tmp = work.tile([P, COLS], fp16, tag="tmp")
            nc.vector.tensor_sub(tmp[:], P1[:], P2[:])

            xc_t = work.tile([P, COLS], fp16, tag="xc_t")
            nc.sync.dma_start(xc_t[:], xc_d[:, b, r0:r0 + RB, :])
            cnew = work.tile([P, COLS], fp16, tag="cnew")
            nc.vector.tensor_add(cnew[:], xc_t[:], tmp[:])
            nc.sync.dma_start(c_d[:, b, r0:r0 + RB, :], cnew[:])

            T = work.tile([P, COLS], fp16, tag="T")        # [tr; ti]
            nc.scalar.activation(T[:], cnew[:], Tanh)

            O = work.tile([P, COLS], fp16, tag="O")        # [or; oi]
            nc.scalar.activation(O[0:HALF, :], yrio[HALF:P, :], Sigmoid,
                                 bias=bias_s[HALF:P, 0:1])
            nc.scalar.activation(O[HALF:P, :], yiio[HALF:P, :], Sigmoid,
                                 bias=bias_s[HALF:P, 1:2])
            O2 = work.tile([P, COLS], fp16, tag="O2")      # [oi; or]
            nc.vector.tensor_copy(O2[0:HALF, :], O[HALF:P, :])
            nc.vector.tensor_copy(O2[HALF:P, :], O[0:HALF, :])

            Q1 = work.tile([P, COLS], fp16, tag="Q1")      # [or*tr ; oi*tr]
            nc.vector.tensor_mul(Q1[0:HALF, :], O[0:HALF, :], T[0:HALF, :])
            nc.vector.tensor_mul(Q1[HALF:P, :], O2[0:HALF, :], T[0:HALF, :])
            Q2 = work.tile([P, COLS], fp16, tag="Q2")      # [oi*ti ; or*ti]
            nc.vector.tensor_mul(Q2[0:HALF, :], O[HALF:P, :], T[HALF:P, :])
            nc.vector.tensor_mul(Q2[HALF:P, :], O2[HALF:P, :], T[HALF:P, :])

            # h = [q1r - q2r ; q1i + q2i] (sign folded into the combine
            # instead of materializing a negated T twin)
            hnew = work.tile([P, COLS], fp16, tag="hnew")
            nc.vector.tensor_sub(hnew[0:HALF, :], Q1[0:HALF, :], Q2[0:HALF, :])
            nc.vector.tensor_add(hnew[HALF:P, :], Q1[HALF:P, :], Q2[HALF:P, :])
            nc.sync.dma_start(h_d[:, b, r0:r0 + RB, :], hnew[:])

        def macro_pair(b, r0, ch, defer=()):
            """Two 8-row macro tiles (r0, r0+8). PE order: gate-c k1+k3,
            io-gates for m0, gate-c k2, io-gates for m1 — so the io psum
            banks get their read slack from the interleaved gate-c groups."""
            r0a, r0b = r0, r0 + RB
            pD = conv_c(ps_D, 'k1', b, ch, r0a, r0b, "pD")
            for b2, ch2 in defer:
                load_z(b2, ch2, after=last_mm[0])
            pE = conv_c(ps_E, 'k3', b, ch, r0a, r0b, "pE")
            Ds = pwork.tile([P, COLS], f32, tag="Ds")
            nc.scalar.activation(Ds[:], pD[:], Copy)
            yrc = pwork.tile([P, COLS], fp16, tag="yrc")   # [yr_c m0 | m1]
            nc.vector.tensor_sub(yrc[:], Ds[:], pE[:])

            pA = conv_io(ps_A, 'k1', b, ch, r0a, "pA")
            pB = conv_io(ps_B, 'k3', b, ch, r0a, "pB")
            pC = conv_io(ps_C, 'k2', b, ch, r0a, "pC")
            yrio0, yiio0 = epilogue_frees(pA, pB, pC)

            pF = conv_c(ps_F, 'k2', b, ch, r0a, r0b, "pF")
            yic = pwork.tile([P, COLS], fp16, tag="yic")   # [yi_c m0 | m1]
            nc.vector.tensor_add(yic[:], Ds[:], pF[:])
            macro_epilogue(b, r0a, 0, yrc, yic, yrio0, yiio0)

            pA = conv_io(ps_A, 'k1', b, ch, r0b, "pA")
            pB = conv_io(ps_B, 'k3', b, ch, r0b, "pB")
            pC = conv_io(ps_C, 'k2', b, ch, r0b, "pC")
            yrio1, yiio1 = epilogue_frees(pA, pB, pC)
            macro_epilogue(b, r0b, HALF, yrc, yic, yrio1, yiio1)

        # (batch, first row, z-chunk) per pair; the next pair's z-chunk load
        # is issued early inside each pair (anchored after the first conv
        # group) so it lands with ~13us of slack
        SCHEDULE = [(0, 0, 'A'), (0, 16, 'B'), (0, 32, 'C'), (0, 48, 'C'),
                    (1, 0, 'A'), (1, 16, 'B'), (1, 32, 'C'), (1, 48, 'C')]
        deferred = {0: [(0, 'B')], 1: [(0, 'C')], 2: [(1, 'A')],
                    3: [(1, 'B')], 4: [(1, 'C')]}
        for pidx, (b, r0, ch) in enumerate(SCHEDULE):
            macro_pair(b, r0, ch, defer=deferred.pop(pidx, ()))

    _split_excess_waits(nc)
    return nc


def _prep_inputs(inputs):
    """Host-side shard + layout prep. Returns per-core in_maps."""
    f16 = np.float16
    x = np.asarray(inputs['x'], np.float32)
    h_prev = np.asarray(inputs['h_prev'], np.float32)
    c_prev = np.asarray(inputs['c_prev'], np.float32)

    xr, xi = x[:, :HALF], x[:, HALF:]
    hr, hi = h_prev[:, :HALF], h_prev[:, HALF:]
    cr, ci = c_prev[:, :HALF], c_prev[:, HALF:]

    # combined conv input, channel-major, zero-padded, fp16: [128, B, 66, 66]
    def prep_z(a, b):
        z = np.concatenate([a, b], axis=1).transpose(1, 0, 2, 3)
        return np.pad(z, ((0, 0), (0, 0), (1, 1), (1, 1))).astype(f16)
    zr_f = np.concatenate([xr, hr], axis=1).transpose(1, 0, 2, 3)
    zi_f = np.concatenate([xi, hi], axis=1).transpose(1, 0, 2, 3)
    pad = ((0, 0), (0, 0), (1, 1), (1, 1))
    zr = np.pad(zr_f, pad).astype(f16)
    zs = np.pad(zr_f + zi_f, pad).astype(f16)
    zd = np.pad(zi_f - zr_f, pad).astype(f16)

    # x (*) c_prev (complex elementwise), channel-major fp16: [128, B, 64, 64]
    xc = np.concatenate([xr * cr - xi * ci, xr * ci + xi * cr],
                        axis=1).transpose(1, 0, 2, 3).astype(f16)

    # gauss-term weights. io pack: [cin 128, 27, cout 128] with gate i in
    # cols 0:64, gate o in 64:128; slots = term-major (k1: 0-8, k3: 9-17,
    # k2: 18-26), tap-minor. c pack: [128, 27, 64].
    def term_w(gn):
        Wr = np.asarray(inputs['Wr_' + gn], np.float32)  # [64, 128, 3, 3]
        Wi = np.asarray(inputs['Wi_' + gn], np.float32)
        return {'k1': Wr + Wi, 'k3': Wi, 'k2': Wr}

    wio = np.empty((NSLOT, P, P), np.float32)
    wc = np.empty((NSLOT, P, HALF), np.float32)
    wi_t, wo_t, wc_t = term_w('i'), term_w('o'), term_w('c')
    for ti, term in enumerate(('k1', 'k3', 'k2')):
        for t in range(9):
            kh, kw = t // 3, t % 3
            s = ti * 9 + t
            wio[s, :, :HALF] = wi_t[term][:, :, kh, kw].T
            wio[s, :, HALF:] = wo_t[term][:, :, kh, kw].T
            wc[s] = wc_t[term][:, :, kh, kw].T
    wio = np.ascontiguousarray(wio.transpose(1, 0, 2)).astype(f16)
    wc = np.ascontiguousarray(wc.transpose(1, 0, 2)).astype(f16)

    def cat(gn, part):
        return np.asarray(inputs[part + '_' + gn], np.float32)
    bias = np.empty((P, 5), np.float32)
    bias[:HALF, 0], bias[HALF:, 0] = cat('i', 'br'), cat('o', 'br')
    bias[:HALF, 1], bias[HALF:, 1] = cat('i', 'bi'), cat('o', 'bi')
    bias[:HALF, 2] = bias[HALF:, 2] = cat('c', 'br')
    bias[:HALF, 3] = bias[HALF:, 3] = cat('c', 'bi')
    bias[:, 4] = -bias[:, 2]

    in_maps = []
    for c in range(N_CORES):
        sl = slice(c * B_CORE, (c + 1) * B_CORE)
        in_maps.append({
            "zr": np.ascontiguousarray(zr[:, sl]),
            "zs": np.ascontiguousarray(zs[:, sl]),
            "zd": np.ascontiguousarray(zd[:, sl]),
            "wio": wio,
            "wc": wc,
            "bias": bias,
            "xc": np.ascontiguousarray(xc[:, sl]),
        })
    return in_maps


def _gather_outputs(results):
    h_full = np.empty((B, P, H, W), np.float32)
    c_full = np.empty((B, P, H, W), np.float32)
    for c in range(N_CORES):
        sl = slice(c * B_CORE, (c + 1) * B_CORE)
        h_full[sl] = results[c]["h_out"].transpose(1, 0, 2, 3).astype(np.float32)
        c_full[sl] = results[c]["c_out"].transpose(1, 0, 2, 3).astype(np.float32)
    return h_full, c_full


def _run(inputs, trace=False, trace_kwargs=None):
    from concourse.bass_utils import run_bass_kernel_spmd

    if "nc" not in _CACHE:
        _CACHE["nc"] = _build_program()
    nc = _CACHE["nc"]
    in_maps = _prep_inputs(inputs)
    r = run_bass_kernel_spmd(nc, in_maps, list(range(N_CORES)),
                             trace=trace, trace_kwargs=trace_kwargs or {})
    return _gather_outputs(r.results), r


def kernel(**inputs):
    (h_full, c_full), _ = _run(inputs)
    return h_full, c_full
